# Initial kernel scaffold
#
"""Trainium2 Bass kernel for nn_Log_GraphConv4d (log-shift-max + 1x1 conv + BN + GeLU).

Math refactor (validated in numpy):
  reference x_j = max_s max(0, x - roll(x, s)) over s in {±1,±3,±7,±15,±31} on H and W
            == x - m,  where m = min(x, all 20 rolls)
  y = W1 @ x + W2 @ x_j            (1x1 conv, channel concat [x, x_j], K=768)
    = (W1+W2) @ x + (-W2) @ m
  BN (eval) + conv bias fold into per-out-channel affine (a, b):
  out = gelu(a * (Wc @ [x; m]) + b)

The min over the 20 rolls + x is computed with a 14-op binary min DAG using
Minkowski-sum composition of shifted sets (per axis: {±3}->{1,±3,7}->{±1,±3,±7}
in 3 ops and {15,31}->{±15,±31} in 2 ops), executed on DVE (bf16, 2x mode) with
two ops offloaded to GPSIMD. A helper copy X1 = roll(x, +1 in w) keeps all
W-axis shift reads 4-byte aligned so DVE stays in its 2x perf mode.

Sharding: data-parallel over batch, 2 samples per core across 8 cores; weights
replicated. No collectives.
"""

import numpy as np
import ml_dtypes

import concourse.bass as bass
import concourse.mybir as mybir
from concourse import bacc
from concourse.tile import TileContext
from concourse.bass_utils import run_bass_kernel_spmd

N_CORES = 8
B, C, H, W = 16, 384, 56, 56
HW = H * W                 # 3136
B_LOC = B // N_CORES       # 2 samples per core
CC = C // 128              # 3 input-channel chunks
OC = 384 // 128            # 3 output-channel chunks
NT = 7                     # n tiles over HW
NF = HW // NT              # 448 columns per matmul (fits one PSUM bank)

BF16 = mybir.dt.bfloat16
F32 = mybir.dt.float32
MIN = mybir.AluOpType.min
GELU = mybir.ActivationFunctionType.Gelu

USE_GPSIMD = False         # GPSIMD tensor_tensor fails walrus ISA check on this toolchain

LAST_RESULTS = None        # BassKernelResults of the most recent run (for test harness)


def _emit_min(eng, out, a, sa, b, sb, axis, L):
    """out = min(roll(a, sa, axis), roll(b, sb, axis)) on [128, cc, H, W] tiles.

    roll(t, s, axis)[i] = t[(i + s) mod L] along `axis` (2 = h, 3 = w).
    Circular wrap is handled by splitting into contiguous AP regions.
    """
    sa %= L
    sb %= L
    cuts = sorted({0, (L - sa) % L, (L - sb) % L})
    for idx, p in enumerate(cuts):
        q = cuts[idx + 1] if idx + 1 < len(cuts) else L
        n = q - p
        alo = (p + sa) % L
        blo = (p + sb) % L
        if axis == 3:
            eng.tensor_tensor(
                out=out[:, :, :, p:q],
                in0=a[:, :, :, alo:alo + n],
                in1=b[:, :, :, blo:blo + n],
                op=MIN,
            )
        else:
            eng.tensor_tensor(
                out=out[:, :, p:q, :],
                in0=a[:, :, alo:alo + n, :],
                in1=b[:, :, blo:blo + n, :],
                op=MIN,
            )


def _build(b_loc=B_LOC, cc=CC, oc=OC, use_gpsimd=USE_GPSIMD, act=GELU, repeat=1):
    nc = bacc.Bacc(None, target_bir_lowering=False)
    kc = 2 * cc

    xd = nc.dram_tensor("x_in", [b_loc, cc, 128, HW], F32, kind="ExternalInput")
    wd = nc.dram_tensor("w_in", [kc, 128, oc * 128], BF16, kind="ExternalInput")
    pd = nc.dram_tensor("p_in", [oc, 128, 2], F32, kind="ExternalInput")
    yd = nc.dram_tensor("y_out", [b_loc, oc, 128, HW], F32, kind="ExternalOutput")

    with TileContext(nc) as tc:
        with tc.tile_pool(name="sb", bufs=1) as pool, \
             tc.tile_pool(name="ps", bufs=6, space="PSUM") as psum:

            # --- prologue: weights + folded BN params (replicated, tiny) ---
            w_sb = pool.tile([128, kc, oc * 128], BF16, tag="w", bufs=1, name="w_sb")
            for k in range(kc):
                nc.sync.dma_start(out=w_sb[:, k, :], in_=wd[k])
            prm = pool.tile([128, oc, 2], F32, tag="prm", bufs=1, name="prm")
            nc.sync.dma_start(out=prm, in_=pd.rearrange("o p t -> p o t"))

            eng_h = nc.gpsimd if use_gpsimd else nc.vector

            for b in [b for _ in range(repeat) for b in range(b_loc)]:
                # --- load + cast f32 -> bf16 during DMA (SWDGE) ---
                X = pool.tile([128, cc, H, W], BF16, tag="X", bufs=2, name="X")
                nc.gpsimd.dma_start(
                    out=X.rearrange("p c h w -> p c (h w)"),
                    in_=xd[b].rearrange("c p f -> p c f"),
                )

                # X1 = roll(X, +1 in w): keeps W-axis shifted reads 4B-aligned.
                X1 = pool.tile([128, cc, H, W], BF16, tag="X1", bufs=1, name="X1")
                nc.scalar.copy(out=X1[:, :, :, 0:W - 1], in_=X[:, :, :, 1:W])
                nc.scalar.copy(out=X1[:, :, :, W - 1:W], in_=X[:, :, :, 0:1])

                # --- 14-op min DAG (offsets validated against reference) ---
                # W-chain (axis=3), via X1 so every shift is even:
                #   A={±3} B={1,±3,7} C={±1,±3,±7} U={15,31} V={±15,±31}
                # H-chain (axis=2) on X directly (row stride is even):
                #   A2,B2,C2,U2,V2 same sets
                # Merges: M1=min(C,V) M2=min(C2,V2) M3=min(M1,M2) m=min(M3,X)
                tA = pool.tile([128, cc, H, W], BF16, tag="t1", bufs=1, name="tA")
                _emit_min(nc.vector, tA, X1, 2, X1, -4, 3, W)              # A
                tA2 = pool.tile([128, cc, H, W], BF16, tag="t4", bufs=1, name="tA2")
                _emit_min(eng_h, tA2, X, 3, X, -3, 2, H)                   # A2 (GPS)
                tB = pool.tile([128, cc, H, W], BF16, tag="t2", bufs=1, name="tB")
                _emit_min(nc.vector, tB, tA, 0, tA, 4, 3, W)               # B
                tU2 = pool.tile([128, cc, H, W], BF16, tag="mp", bufs=2, name="tU2")
                _emit_min(eng_h, tU2, X, 15, X, 31, 2, H)                  # U2 (GPS)
                tC = pool.tile([128, cc, H, W], BF16, tag="t1", bufs=1, name="tC")
                _emit_min(nc.vector, tC, tB, 0, tB, -4, 3, W)              # C
                tU = pool.tile([128, cc, H, W], BF16, tag="t2", bufs=1, name="tU")
                _emit_min(nc.vector, tU, X1, 14, X1, 30, 3, W)             # U
                tV = pool.tile([128, cc, H, W], BF16, tag="t3", bufs=1, name="tV")
                _emit_min(nc.vector, tV, tU, 0, tU, -46, 3, W)             # V
                tB2 = pool.tile([128, cc, H, W], BF16, tag="t2", bufs=1, name="tB2")
                _emit_min(nc.vector, tB2, tA2, 0, tA2, 4, 2, H)            # B2
                tC2 = pool.tile([128, cc, H, W], BF16, tag="t4", bufs=1, name="tC2")
                _emit_min(nc.vector, tC2, tB2, 0, tB2, -4, 2, H)           # C2
                tM1 = pool.tile([128, cc, H, W], BF16, tag="t2", bufs=1, name="tM1")
                _emit_min(nc.vector, tM1, tC, 0, tV, 0, 3, W)              # M1
                tV2 = pool.tile([128, cc, H, W], BF16, tag="t1", bufs=1, name="tV2")
                _emit_min(nc.vector, tV2, tU2, 0, tU2, -46, 2, H)          # V2
                tM2 = pool.tile([128, cc, H, W], BF16, tag="t3", bufs=1, name="tM2")
                _emit_min(nc.vector, tM2, tC2, 0, tV2, 0, 2, H)            # M2
                tM3 = pool.tile([128, cc, H, W], BF16, tag="t4", bufs=1, name="tM3")
                _emit_min(nc.vector, tM3, tM1, 0, tM2, 0, 3, W)            # M3
                mp = pool.tile([128, cc, H, W], BF16, tag="mp", bufs=2, name="mp")
                _emit_min(nc.vector, mp, tM3, 0, X, 0, 3, W)               # m'

                # --- matmul K=2C contraction + fused BN-affine + GeLU ---
                Xf = X.rearrange("p c h w -> p c (h w)")
                Mf = mp.rearrange("p c h w -> p c (h w)")
                for o in range(oc):
                    y_sb = pool.tile([128, HW], F32, tag="y", bufs=2, name="y_sb")
                    for n in range(NT):
                        pst = psum.tile([128, NF], F32, tag="ps", name="pst")
                        for k in range(kc):
                            src = Xf if k < cc else Mf
                            nc.tensor.matmul(
                                pst,
                                lhsT=w_sb[:, k, o * 128:(o + 1) * 128],
                                rhs=src[:, k % cc, n * NF:(n + 1) * NF],
                                start=(k == 0),
                                stop=(k == kc - 1),
                            )
                        nc.scalar.activation(
                            out=y_sb[:, n * NF:(n + 1) * NF],
                            in_=pst,
                            func=act,
                            bias=prm[:, o, 1:2],
                            scale=prm[:, o, 0:1],
                        )
                    nc.sync.dma_start(out=yd[b, o], in_=y_sb)
    nc.finalize()  # Bacc: wait-splitting, reg alloc, event sems — required by walrus
    return nc


_CACHE = {}


def _get_program():
    if "nc" not in _CACHE:
        _CACHE["nc"] = _build()
    return _CACHE["nc"]


def kernel(x, conv_w, conv_b, bn_scale, bn_bias, bn_mean, bn_var, _trace=False):
    global LAST_RESULTS
    x = np.asarray(x, dtype=np.float32)
    conv_w = np.asarray(conv_w, dtype=np.float32)
    conv_b = np.asarray(conv_b, dtype=np.float32)
    bn_scale = np.asarray(bn_scale, dtype=np.float32)
    bn_bias = np.asarray(bn_bias, dtype=np.float32)
    bn_mean = np.asarray(bn_mean, dtype=np.float32)
    bn_var = np.asarray(bn_var, dtype=np.float32)

    # host-side weight/param folding
    Wm = conv_w[:, :, 0, 0]                      # [384, 768]
    W1, W2 = Wm[:, :C], Wm[:, C:]
    wT = np.concatenate([(W1 + W2).T, (-W2).T], axis=0)   # [768, 384], rows = K
    wd_arr = np.ascontiguousarray(
        wT.reshape(2 * CC, 128, OC * 128).astype(ml_dtypes.bfloat16)
    )
    inv = 1.0 / np.sqrt(bn_var + 1e-5)
    a = (inv * bn_scale).astype(np.float32)               # per-channel scale
    b_aff = ((conv_b - bn_mean) * a + bn_bias).astype(np.float32)
    prm_arr = np.ascontiguousarray(
        np.stack([a.reshape(OC, 128), b_aff.reshape(OC, 128)], axis=-1)
    )                                                      # [3, 128, 2]

    xs = x.reshape(B, CC, 128, HW)
    in_maps = []
    for core in range(N_CORES):
        shard = np.ascontiguousarray(xs[core * B_LOC:(core + 1) * B_LOC])
        in_maps.append({"x_in": shard, "w_in": wd_arr, "p_in": prm_arr})

    nc = _get_program()
    res = run_bass_kernel_spmd(nc, in_maps, core_ids=list(range(N_CORES)))
    LAST_RESULTS = res
    y = np.concatenate([r["y_out"] for r in res.results], axis=0)
    return y.reshape(B, C, H, W)



# revision 2
# speedup vs baseline: 1.9371x; 1.9371x over previous
"""Trainium2 Bass kernel for nn_Log_GraphConv4d (log-shift-max + 1x1 conv + BN + GeLU).

Math refactor (validated in numpy):
  reference x_j = max_s max(0, x - roll(x, s)) over s in {±1,±3,±7,±15,±31} on H and W
            == x - m,  where m = min(x, all 20 rolls)
  y = W1 @ x + W2 @ x_j            (1x1 conv, channel concat [x, x_j], K=768)
    = (W1+W2) @ x + (-W2) @ m
  BN (eval) + conv bias fold into per-out-channel affine (a, b):
  out = gelu(a * (Wc @ [x; m]) + b)

The min over the 20 rolls + x is computed with a 14-op binary min DAG using
Minkowski-sum composition of shifted sets (per axis: {±3}->{1,±3,7}->{±1,±3,±7}
in 3 ops and {15,31}->{±15,±31} in 2 ops), executed on DVE (bf16, 2x mode) with
two ops offloaded to GPSIMD. A helper copy X1 = roll(x, +1 in w) keeps all
W-axis shift reads 4-byte aligned so DVE stays in its 2x perf mode.

Sharding: data-parallel over batch, 2 samples per core across 8 cores; weights
replicated. No collectives.
"""

import numpy as np
import ml_dtypes

import concourse.bass as bass
import concourse.mybir as mybir
from concourse import bacc
from concourse.tile import TileContext
from concourse.bass_utils import run_bass_kernel_spmd

N_CORES = 8
B, C, H, W = 16, 384, 56, 56
HW = H * W                 # 3136
B_LOC = B // N_CORES       # 2 samples per core
CC = C // 128              # 3 input-channel chunks
OC = 384 // 128            # 3 output-channel chunks
NT = 7                     # n tiles over HW
NF = HW // NT              # 448 columns per matmul (fits one PSUM bank)

BF16 = mybir.dt.bfloat16
F32 = mybir.dt.float32
MIN = mybir.AluOpType.min
GELU = mybir.ActivationFunctionType.Gelu

USE_GPSIMD = False         # GPSIMD tensor_tensor fails walrus ISA check on this toolchain

LAST_RESULTS = None        # BassKernelResults of the most recent run (for test harness)


def _emit_min(eng, out, a, sa, b, sb, axis, L):
    """out = min(roll(a, sa, axis), roll(b, sb, axis)) on [128, cc, H, W] tiles.

    roll(t, s, axis)[i] = t[(i + s) mod L] along `axis` (2 = h, 3 = w).
    Circular wrap is handled by splitting into contiguous AP regions.
    """
    sa %= L
    sb %= L
    cuts = sorted({0, (L - sa) % L, (L - sb) % L})
    for idx, p in enumerate(cuts):
        q = cuts[idx + 1] if idx + 1 < len(cuts) else L
        n = q - p
        alo = (p + sa) % L
        blo = (p + sb) % L
        if axis == 3:
            eng.tensor_tensor(
                out=out[:, :, :, p:q],
                in0=a[:, :, :, alo:alo + n],
                in1=b[:, :, :, blo:blo + n],
                op=MIN,
            )
        else:
            eng.tensor_tensor(
                out=out[:, :, p:q, :],
                in0=a[:, :, alo:alo + n, :],
                in1=b[:, :, blo:blo + n, :],
                op=MIN,
            )


def _build(b_loc=B_LOC, cc=CC, oc=OC, use_gpsimd=USE_GPSIMD, act=GELU, repeat=1):
    nc = bacc.Bacc(None, target_bir_lowering=False)
    kc = 2 * cc

    xd = nc.dram_tensor("x_in", [b_loc, cc, 128, HW], F32, kind="ExternalInput")
    wd = nc.dram_tensor("w_in", [kc, 128, oc * 128], BF16, kind="ExternalInput")
    pd = nc.dram_tensor("p_in", [oc, 128, 2], F32, kind="ExternalInput")
    yd = nc.dram_tensor("y_out", [b_loc, oc, 128, HW], F32, kind="ExternalOutput")

    with TileContext(nc) as tc:
        with tc.tile_pool(name="sb", bufs=1) as pool, \
             tc.tile_pool(name="ps", bufs=6, space="PSUM") as psum:

            # --- prologue: weights + folded BN params (replicated, tiny) ---
            w_sb = pool.tile([128, kc, oc * 128], BF16, tag="w", bufs=1, name="w_sb")
            for k in range(kc):
                nc.sync.dma_start(out=w_sb[:, k, :], in_=wd[k])
            prm = pool.tile([128, oc, 2], F32, tag="prm", bufs=1, name="prm")
            nc.sync.dma_start(out=prm, in_=pd.rearrange("o p t -> p o t"))

            eng_h = nc.gpsimd if use_gpsimd else nc.vector

            for b in [b for _ in range(repeat) for b in range(b_loc)]:
                # --- load + cast f32 -> bf16 during DMA (SWDGE) ---
                X = pool.tile([128, cc, H, W], BF16, tag="X", bufs=2, name="X")
                nc.gpsimd.dma_start(
                    out=X.rearrange("p c h w -> p c (h w)"),
                    in_=xd[b].rearrange("c p f -> p c f"),
                )

                # X1 = roll(X, +1 in w): keeps W-axis shifted reads 4B-aligned.
                X1 = pool.tile([128, cc, H, W], BF16, tag="X1", bufs=1, name="X1")
                nc.scalar.copy(out=X1[:, :, :, 0:W - 1], in_=X[:, :, :, 1:W])
                nc.scalar.copy(out=X1[:, :, :, W - 1:W], in_=X[:, :, :, 0:1])

                # --- 14-op min DAG (offsets validated against reference) ---
                # W-chain (axis=3), via X1 so every shift is even:
                #   A={±3} B={1,±3,7} C={±1,±3,±7} U={15,31} V={±15,±31}
                # H-chain (axis=2) on X directly (row stride is even):
                #   A2,B2,C2,U2,V2 same sets
                # Merges: M1=min(C,V) M2=min(C2,V2) M3=min(M1,M2) m=min(M3,X)
                tA = pool.tile([128, cc, H, W], BF16, tag="t1", bufs=1, name="tA")
                _emit_min(nc.vector, tA, X1, 2, X1, -4, 3, W)              # A
                tA2 = pool.tile([128, cc, H, W], BF16, tag="t4", bufs=1, name="tA2")
                _emit_min(eng_h, tA2, X, 3, X, -3, 2, H)                   # A2 (GPS)
                tB = pool.tile([128, cc, H, W], BF16, tag="t2", bufs=1, name="tB")
                _emit_min(nc.vector, tB, tA, 0, tA, 4, 3, W)               # B
                tU2 = pool.tile([128, cc, H, W], BF16, tag="mp", bufs=2, name="tU2")
                _emit_min(eng_h, tU2, X, 15, X, 31, 2, H)                  # U2 (GPS)
                tC = pool.tile([128, cc, H, W], BF16, tag="t1", bufs=1, name="tC")
                _emit_min(nc.vector, tC, tB, 0, tB, -4, 3, W)              # C
                tU = pool.tile([128, cc, H, W], BF16, tag="t2", bufs=1, name="tU")
                _emit_min(nc.vector, tU, X1, 14, X1, 30, 3, W)             # U
                tV = pool.tile([128, cc, H, W], BF16, tag="t3", bufs=1, name="tV")
                _emit_min(nc.vector, tV, tU, 0, tU, -46, 3, W)             # V
                tB2 = pool.tile([128, cc, H, W], BF16, tag="t2", bufs=1, name="tB2")
                _emit_min(nc.vector, tB2, tA2, 0, tA2, 4, 2, H)            # B2
                tC2 = pool.tile([128, cc, H, W], BF16, tag="t4", bufs=1, name="tC2")
                _emit_min(nc.vector, tC2, tB2, 0, tB2, -4, 2, H)           # C2
                tM1 = pool.tile([128, cc, H, W], BF16, tag="t2", bufs=1, name="tM1")
                _emit_min(nc.vector, tM1, tC, 0, tV, 0, 3, W)              # M1
                tV2 = pool.tile([128, cc, H, W], BF16, tag="t1", bufs=1, name="tV2")
                _emit_min(nc.vector, tV2, tU2, 0, tU2, -46, 2, H)          # V2
                tM2 = pool.tile([128, cc, H, W], BF16, tag="t3", bufs=1, name="tM2")
                _emit_min(nc.vector, tM2, tC2, 0, tV2, 0, 2, H)            # M2
                tM3 = pool.tile([128, cc, H, W], BF16, tag="t4", bufs=1, name="tM3")
                _emit_min(nc.vector, tM3, tM1, 0, tM2, 0, 3, W)            # M3
                mp = pool.tile([128, cc, H, W], BF16, tag="mp", bufs=2, name="mp")
                _emit_min(nc.vector, mp, tM3, 0, X, 0, 3, W)               # m'

                # --- matmul K=2C contraction + fused BN-affine + GeLU ---
                Xf = X.rearrange("p c h w -> p c (h w)")
                Mf = mp.rearrange("p c h w -> p c (h w)")
                for o in range(oc):
                    y_sb = pool.tile([128, HW], F32, tag="y", bufs=2, name="y_sb")
                    for n in range(NT):
                        pst = psum.tile([128, NF], F32, tag="ps", name="pst")
                        for k in range(kc):
                            src = Xf if k < cc else Mf
                            nc.tensor.matmul(
                                pst,
                                lhsT=w_sb[:, k, o * 128:(o + 1) * 128],
                                rhs=src[:, k % cc, n * NF:(n + 1) * NF],
                                start=(k == 0),
                                stop=(k == kc - 1),
                            )
                        nc.scalar.activation(
                            out=y_sb[:, n * NF:(n + 1) * NF],
                            in_=pst,
                            func=act,
                            bias=prm[:, o, 1:2],
                            scale=prm[:, o, 0:1],
                        )
                    nc.sync.dma_start(out=yd[b, o], in_=y_sb)
    nc.finalize()  # Bacc: wait-splitting, reg alloc, event sems — required by walrus
    return nc


_CACHE = {}


def _get_program():
    if "nc" not in _CACHE:
        _CACHE["nc"] = _build()
    return _CACHE["nc"]


def kernel(x, conv_w, conv_b, bn_scale, bn_bias, bn_mean, bn_var, _trace=False):
    global LAST_RESULTS
    x = np.asarray(x, dtype=np.float32)
    conv_w = np.asarray(conv_w, dtype=np.float32)
    conv_b = np.asarray(conv_b, dtype=np.float32)
    bn_scale = np.asarray(bn_scale, dtype=np.float32)
    bn_bias = np.asarray(bn_bias, dtype=np.float32)
    bn_mean = np.asarray(bn_mean, dtype=np.float32)
    bn_var = np.asarray(bn_var, dtype=np.float32)

    # host-side weight/param folding
    Wm = conv_w[:, :, 0, 0]                      # [384, 768]
    W1, W2 = Wm[:, :C], Wm[:, C:]
    wT = np.concatenate([(W1 + W2).T, (-W2).T], axis=0)   # [768, 384], rows = K
    wd_arr = np.ascontiguousarray(
        wT.reshape(2 * CC, 128, OC * 128).astype(ml_dtypes.bfloat16)
    )
    inv = 1.0 / np.sqrt(bn_var + 1e-5)
    a = (inv * bn_scale).astype(np.float32)               # per-channel scale
    b_aff = ((conv_b - bn_mean) * a + bn_bias).astype(np.float32)
    prm_arr = np.ascontiguousarray(
        np.stack([a.reshape(OC, 128), b_aff.reshape(OC, 128)], axis=-1)
    )                                                      # [3, 128, 2]

    xs = x.reshape(B, CC, 128, HW)
    in_maps = []
    for core in range(N_CORES):
        shard = np.ascontiguousarray(xs[core * B_LOC:(core + 1) * B_LOC])
        in_maps.append({"x_in": shard, "w_in": wd_arr, "p_in": prm_arr})

    nc = _get_program()
    res = run_bass_kernel_spmd(nc, in_maps, core_ids=list(range(N_CORES)),
                               trace=_trace)
    LAST_RESULTS = res
    y = np.concatenate([r["y_out"] for r in res.results], axis=0)
    return y.reshape(B, C, H, W)

